# revision 1
# baseline (speedup 1.0000x reference)
"""Trainium2 Bass kernel for a 2-layer dense GAT (nn_GAT_87144886436203).

Sharding: row-shard the N=4096 nodes across 8 NeuronCores (512 rows each).
Each core computes attention scores for its row block against all N columns,
with the contraction axis j kept on SBUF partitions so `att @ Wh` needs no
transposes. Softmax normalization rides a ones-column appended to Wh (the
row-sum falls out of the same matmul; the adjacency
mask is applied multiplicatively after exp). One small AllGather moves the per-core layer-2
node features (Wh2 | ones | g2) between the layers.

Score pipeline per group of 4 [128 j, 512 i] tiles (working dtype fp16):
    u  = leaky_relu(F_bcast + g[j])  (ACT Prelu with per-partition bias -- or,
                                      on ~2/3 of groups, DVE ts-add + mul + max
                                      to balance the ACT and DVE engines)
    p  = exp(u)                      (ACT Exp, one 2048-wide op)
    pm = p * adj01                   (DVE tensor_tensor; exact zeros off-edges)
    psum[65,512] += Whb[j,:].T @ pm  (PE; the ones-column yields the row sums)
F_bcast tiles are filled by stride-0 broadcast DMAs from a small DRAM bounce
of the on-device f/g projections.
"""

import numpy as np
import ml_dtypes

import concourse.bass as bass
import concourse.bacc as bacc
import concourse.tile as tile
import concourse.mybir as mybir
from concourse import masks
from concourse.bass_utils import run_bass_kernel_spmd

F16 = mybir.dt.float16
F32 = mybir.dt.float32
NPF16 = ml_dtypes.float16 if hasattr(ml_dtypes, "float16") else np.float16

NCORES = 8
N = 4096            # nodes
K = 512             # input feature dim (= NFEAT)
H = 8               # heads (layer 1)
D = 64              # per-head hidden (= NHID = NCLASS)
DALL = H * D        # 512
R = N // NCORES     # 512 rows per core
JC = N // 128       # 32 j-chunks
G = 4               # j-chunks per group (free dim 2048 for the big ops)
NG = JC // G        # 8 groups
AUG1 = D + 1        # 65: [Wh_h | ones]
W1S = H * AUG1      # 520 per-chunk stride of the layer-1 weight buffer
AUG2 = D + 2        # 66: [Wh2 | ones | g2]
ALPHA = 0.2
BIG = 100.0

# Fraction of the 72 attention groups whose leaky_relu runs on the DVE
# (mul+max) instead of ACT (Prelu). Tuned to balance ACT vs DVE load.
N_UNITS = H + 1
NGROUPS = N_UNITS * NG
D_COUNT = 42          # leaky_relu on DVE for this many of the 72 groups
PMASK_COUNT = 0      # adjacency mask-multiply on GPSIMD for this many


def _bres(i, count):
    return (i * count) // NGROUPS != ((i + 1) * count) // NGROUPS


def _on_dve(unit, g):
    # during the DMA-starved ramp (first groups of the lead units) favor ACT
    if unit < 4 and g < 2:
        return False
    return _bres(unit * NG + g, D_COUNT)


def _mask_on_pool(unit, g):
    return _bres(unit * NG + g, PMASK_COUNT)

_CACHE = {}


# --------------------------------------------------------------------------- #
# device program
# --------------------------------------------------------------------------- #

def _build(emulate_collective=False):
    """Build (and compile) the per-core Bass program.

    emulate_collective=True builds a single-core variant with the AllGather
    replaced by local DMAs of the same volume -- used for cost-model timing.
    """
    nc = bacc.Bacc(
        "TRN2",
        target_bir_lowering=False,
        debug=False,
        num_devices=1 if emulate_collective else NCORES,
    )

    xT = nc.dram_tensor("xT", [K, N], F16, kind="ExternalInput")
    xrT = nc.dram_tensor("xrT", [K, R], F16, kind="ExternalInput")
    adjB = nc.dram_tensor("adjB", [N, R], F16, kind="ExternalInput")
    W_all = nc.dram_tensor("W_all", [K, DALL], F16, kind="ExternalInput")
    wa = nc.dram_tensor("wa", [K, 2 * H], F16, kind="ExternalInput")
    W_out = nc.dram_tensor("W_out", [DALL, D], F16, kind="ExternalInput")
    wa2 = nc.dram_tensor("wa2", [DALL, 2], F16, kind="ExternalInput")
    out = nc.dram_tensor("out", [R, D], F32, kind="ExternalOutput")

    with tile.TileContext(nc) as tc:
        _emit(nc, tc, locals(), emulate_collective)

    nc.compile()
    return nc


def _emit(nc, tc, io, emulate_collective):
    xT, xrT, adjB, W_all, wa, W_out, wa2, out = (
        io["xT"], io["xrT"], io["adjB"], io["W_all"], io["wa"],
        io["W_out"], io["wa2"], io["out"],
    )
    AT = mybir.AluOpType
    AF = mybir.ActivationFunctionType

    from contextlib import ExitStack
    with ExitStack() as ctx:
        res = ctx.enter_context(tc.tile_pool(name="res", bufs=1))
        psum = ctx.enter_context(tc.tile_pool(name="psum", bufs=4, space="PSUM"))
        ppool = ctx.enter_context(tc.tile_pool(name="ppool", bufs=4, space="PSUM"))
        work = ctx.enter_context(tc.tile_pool(name="work", bufs=3))
        work2 = ctx.enter_context(tc.tile_pool(name="work2", bufs=2))
        tpool = ctx.enter_context(tc.tile_pool(name="tpool", bufs=4))
        small = ctx.enter_context(tc.tile_pool(name="small", bufs=4))
        rpool = ctx.enter_context(tc.tile_pool(name="rpool", bufs=2))
        dram = ctx.enter_context(tc.tile_pool(name="dram", bufs=1, space="DRAM"))

        # ---- resident SBUF tensors (chunk-major [128, n_chunks*width]) ---- #
        xT_sb = res.tile([128, 4 * N], F16, tag="xT")
        xrT_sb = res.tile([128, 4 * R], F16, tag="xrT")
        adjB_sb = res.tile([128, JC * R], F16, tag="adjB")
        W_all_sb = res.tile([128, 4 * DALL], F16, tag="W_all")
        wa_sb = res.tile([128, 4 * 2 * H], F16, tag="wa")
        W_out_sb = res.tile([128, 4 * D], F16, tag="W_out")
        wa2_sb = res.tile([128, 4 * 2], F16, tag="wa2")
        whb_sb = res.tile([128, JC * W1S], F16, tag="whb")       # [Wh_h|1]x8 per chunk
        fg_sb = res.tile([128, JC * 2 * H], F32, tag="fg")       # f/g all nodes
        hcatT_sb = res.tile([128, 4 * R], F16, tag="hcatT")      # [DALL, R] chunk-major
        whb2_sb = res.tile([128, JC * AUG2], F16, tag="whb2")    # gathered layer-2
        fg2_sb = res.tile([2, R], F32, tag="fg2")
        ones_sb = res.tile([1, 128], F32, tag="ones")
        ident_sb = res.tile([64, 64], F32, tag="ident")
        out_sb = res.tile([128, 4 * D], F32, tag="out_sb")

        def chunked(dram_t, width):
            return dram_t.ap().rearrange("(c p) w -> p c w", p=128)

        def chunked_sb(sb_ap, width):
            return sb_ap.rearrange("p (c w) -> p c w", w=width)

        def load(sb_tile, dram_t, width, split=1, split_free=1):
            dst = chunked_sb(sb_tile[:], width)
            src = chunked(dram_t, width)
            nch = dst.shape[1]
            step = max(1, nch // split)
            fstep = max(1, width // split_free)
            for lo in range(0, nch, step):
                hi = min(nch, lo + step)
                for flo in range(0, width, fstep):
                    fhi = min(width, flo + fstep)
                    nc.sync.dma_start(
                        dst[:, lo:hi, flo:fhi], src[:, lo:hi, flo:fhi])

        # ---- phase 0: loads + constants ---- #
        # adjB/fg feed the score pipeline chunk-by-chunk: split those loads so
        # the first groups start without waiting for the full 4MB transfers.
        load(xrT_sb, xrT, R)
        load(wa_sb, wa, 2 * H)
        load(W_all_sb, W_all, DALL)
        load(adjB_sb, adjB, R, split=8)
        load(xT_sb, xT, N, split_free=16)
        load(W_out_sb, W_out, D)
        load(wa2_sb, wa2, 2)
        nc.vector.memset(ones_sb[:], 1.0)
        masks.make_identity(nc, ident_sb[:])


        # ---- phase 1 helpers: per-chunk Wh/fg prep ---- #
        def prep_chunk(jc):
            # memset the whole chunk; Wh copies then overwrite the data
            # columns, leaving 1.0 in each head's ones-column
            nc.gpsimd.memset(whb_sb[:, jc * W1S:(jc + 1) * W1S], 1.0)
            pw = psum.tile([128, DALL], F32, tag="bank")
            pf = psum.tile([128, 2 * H], F32, tag="bank")
            for kc in range(4):
                lhsT = xT_sb[:, kc * N + jc * 128: kc * N + (jc + 1) * 128]
                nc.tensor.matmul(
                    pw[:], lhsT, W_all_sb[:, kc * DALL:(kc + 1) * DALL],
                    start=(kc == 0), stop=(kc == 3),
                )
                nc.tensor.matmul(
                    pf[:], lhsT, wa_sb[:, kc * 2 * H:(kc + 1) * 2 * H],
                    start=(kc == 0), stop=(kc == 3),
                )
            dst = whb_sb[:, jc * W1S:(jc + 1) * W1S].rearrange(
                "p (h x) -> p h x", x=AUG1)[:, :, 0:D]
            nc.vector.tensor_copy(dst, pw.rearrange("p (h x) -> p h x", x=D))
            nc.vector.tensor_copy(fg_sb[:, jc * 2 * H:(jc + 1) * 2 * H], pf[:])

        def emit_fg_rows():
            pfr = psum.tile([16, R], F32, tag="bank")
            for kc in range(4):
                nc.tensor.matmul(
                    pfr[:], wa_sb[:, kc * 2 * H:(kc + 1) * 2 * H],
                    xrT_sb[:, kc * R:(kc + 1) * R],
                    start=(kc == 0), stop=(kc == 3),
                )
            fgr16 = res.tile([16, R], F16, tag="fgr16")
            nc.vector.tensor_copy(fgr16[:], pfr[:])
            fgb_d = dram.tile([16, R], F16, tag="fgb")
            nc.gpsimd.dma_start(fgb_d[:], fgr16[:])
            return fgb_d

        # ---- attention unit (group-at-a-time emission) ---- #
        def unit_start(f_row_dram, n_aug):
            pout = ppool.tile([n_aug, R], F32, tag="pout")
            frep = tpool.tile([128, G * R], F16, tag="frep")
            nc.gpsimd.dma_start(
                frep[:].rearrange("p (c w) -> p c w", w=R),
                f_row_dram.broadcast_to([128, R]).unsqueeze(1)
                .broadcast_to([128, G, R]))
            return pout, frep

        def unit_group(pout, frep, lhsT_of, g_of, g, on_dve, mask_pool=False):
            u = work.tile([128, G * R], F16, tag="u")
            if on_dve:
                # s = f + g on DVE (4x ts), lrelu via mul+max
                s = work2.tile([128, G * R], F16, tag="s1")
                for c in range(G):
                    jc = g * G + c
                    nc.vector.tensor_scalar(
                        s[:, c * R:(c + 1) * R],
                        frep[:, c * R:(c + 1) * R],
                        g_of(jc), None, AT.add,
                    )
                t = work2.tile([128, G * R], F16, tag="t")
                nc.vector.tensor_scalar(t[:], s[:], ALPHA, None, AT.mult)
                nc.vector.tensor_tensor(u[:], s[:], t[:], AT.max)
            else:
                # lrelu(f + g) in one ACT pass per chunk (bias = g)
                for c in range(G):
                    jc = g * G + c
                    nc.scalar.activation(
                        u[:, c * R:(c + 1) * R],
                        frep[:, c * R:(c + 1) * R],
                        AF.Prelu, bias=g_of(jc), alpha=ALPHA)
            p = work.tile([128, G * R], F16, tag="p")
            nc.scalar.activation(p[:], u[:], AF.Exp)
            pm = work.tile([128, G * R], F16, tag="pm")
            eng = nc.gpsimd if mask_pool else nc.vector
            eng.tensor_tensor(
                pm[:], p[:], adjB_sb[:, g * G * R:(g + 1) * G * R], AT.mult)
            for c in range(G):
                jc = g * G + c
                nc.tensor.matmul(
                    pout[:], lhsT_of(jc), pm[:, c * R:(c + 1) * R],
                    start=(jc == 0), stop=(jc == JC - 1),
                )

        def attention_unit(unit, lhsT_of, g_of, f_row_dram, n_aug):
            pout, frep = unit_start(f_row_dram, n_aug)
            for g in range(NG):
                unit_group(pout, frep, lhsT_of, g_of, g, _on_dve(unit, g),
                           _mask_on_pool(unit, g))
            return pout

        def epilogue(pout, dst_ap, dst_f32):
            """dst = elu(att_out / rowsum) written to dst_ap ([64, R])."""
            dt = F32 if dst_f32 else F16
            recip = rpool.tile([1, R], F32, tag="recip")
            nc.vector.reciprocal(recip[:], pout[D:D + 1, :])
            pr = psum.tile([D, R], F32, tag="bank")
            nc.tensor.matmul(pr[:], ones_sb[0:1, 0:D], recip[:])
            rsb = small.tile([D, R], F32, tag="ep")
            nc.any.tensor_copy(rsb[:], pr[:])
            hl = small.tile([D, R], dt, tag="ep")
            nc.vector.tensor_tensor(hl[:], pout[0:D, :], rsb[:], AT.mult)
            # elu(x) = max(x,0) + min(exp(x),1) - 1   (exp monotone)
            q = small.tile([D, R], dt, tag="ep")
            nc.scalar.activation(q[:], hl[:], AF.Exp)
            t1 = small.tile([D, R], dt, tag="ep")
            nc.vector.tensor_scalar(t1[:], q[:], 1.0, -1.0, AT.min, AT.add)
            t2 = small.tile([D, R], dt, tag="ep")
            nc.vector.tensor_scalar(t2[:], hl[:], 0.0, None, AT.max)
            nc.vector.tensor_tensor(dst_ap, t1[:], t2[:], AT.add)

        # ---- phases 1+2 interleaved: chunk prep rides along with head 0 ---- #
        def l1_args(h):
            return (
                lambda jc, h=h: whb_sb[
                    :, jc * W1S + h * AUG1: jc * W1S + (h + 1) * AUG1],
                lambda jc, h=h: fg_sb[
                    :, jc * 2 * H + 2 * h + 1: jc * 2 * H + 2 * h + 2],
            )

        fgb_d = emit_fg_rows()
        NLEAD = 4   # heads interleaved with the chunk prep
        lead = []
        for h in range(NLEAD):
            lhsTh, gh = l1_args(h)
            pout, frep = unit_start(fgb_d[2 * h:2 * h + 1, :], AUG1)
            lead.append((h, pout, frep, lhsTh, gh))
        for jc in range(JC):
            prep_chunk(jc)
            if jc % G == G - 1:
                for (h, pout, frep, lhsTh, gh) in lead:
                    unit_group(pout, frep, lhsTh, gh, jc // G,
                               _on_dve(h, jc // G), _mask_on_pool(h, jc // G))
        for h in range(NLEAD):
            pout = lead[h][1]
            kc, po = h // 2, (h % 2) * D
            epilogue(pout, hcatT_sb[po:po + D, kc * R:(kc + 1) * R],
                     dst_f32=False)

        for hp in range(NLEAD, H, 2):
            pair = []
            for h in (hp, hp + 1):
                lhsTh, gh = l1_args(h)
                pout, frep = unit_start(fgb_d[2 * h:2 * h + 1, :], AUG1)
                pair.append((h, pout, frep, lhsTh, gh))
            for g in range(NG):
                for (h, pout, frep, lhsTh, gh) in pair:
                    unit_group(pout, frep, lhsTh, gh, g,
                               _on_dve(h, g), _mask_on_pool(h, g))
            for (h, pout, frep, lhsTh, gh) in pair:
                kc, po = h // 2, (h % 2) * D
                epilogue(
                    pout,
                    hcatT_sb[po:po + D, kc * R:(kc + 1) * R],
                    dst_f32=False,
                )

        # ---- phase 3: layer-2 prep + allgather ---- #
        gt_sb = res.tile([128, 4 * AUG2], F16, tag="gt")
        nc.vector.memset(gt_sb[:], 1.0)   # ones column comes for free
        for ib in range(4):
            pw2 = psum.tile([128, D], F32, tag="bank")
            pg2 = psum.tile([128, 2], F32, tag="bank")
            for kc in range(4):
                lhsT = hcatT_sb[:, kc * R + ib * 128: kc * R + (ib + 1) * 128]
                nc.tensor.matmul(pw2[:], lhsT, W_out_sb[:, kc * D:(kc + 1) * D],
                                 start=(kc == 0), stop=(kc == 3))
                nc.tensor.matmul(pg2[:], lhsT, wa2_sb[:, kc * 2:(kc + 1) * 2],
                                 start=(kc == 0), stop=(kc == 3))
            nc.vector.tensor_copy(gt_sb[:, ib * AUG2: ib * AUG2 + D], pw2[:])
            nc.vector.tensor_copy(
                gt_sb[:, ib * AUG2 + D + 1: ib * AUG2 + D + 2], pg2[:, 1:2])

        pfg2 = psum.tile([2, R], F32, tag="bank")
        for kc in range(4):
            nc.tensor.matmul(pfg2[:], wa2_sb[:, kc * 2:(kc + 1) * 2],
                             hcatT_sb[:, kc * R:(kc + 1) * R],
                             start=(kc == 0), stop=(kc == 3))
        nc.vector.tensor_copy(fg2_sb[:], pfg2[:])
        fg2_16 = res.tile([2, R], F16, tag="fg2_16")
        nc.vector.tensor_copy(fg2_16[:], fg2_sb[:])
        fgb2_d = dram.tile([2, R], F16, tag="fgb2")
        nc.sync.dma_start(fgb2_d[:], fg2_16[:])

        cc_in = dram.tile([R, AUG2], F16, tag="cc_in")
        cc_space = {} if emulate_collective else {"addr_space": "Shared"}
        cc_out = dram.tile([N, AUG2], F16, tag="cc_out", **cc_space)
        nc.sync.dma_start(
            cc_in[:].rearrange("(c p) w -> p c w", p=128),
            chunked_sb(gt_sb[:], AUG2))
        if emulate_collective:
            for c in range(NCORES):
                nc.sync.dma_start(cc_out[c * R:(c + 1) * R, :], cc_in[:])
        else:
            nc.gpsimd.collective_compute(
                "AllGather", mybir.AluOpType.bypass,
                replica_groups=[list(range(NCORES))],
                ins=[cc_in.opt()], outs=[cc_out.opt()],
            )
        whb2_ch = chunked_sb(whb2_sb[:], AUG2)
        cc_out_ch = cc_out[:].rearrange("(c p) w -> p c w", p=128)
        g2_sb = res.tile([128, JC], F32, tag="g2")
        g2_ch = g2_sb[:].rearrange("p (c w) -> p c w", w=1)
        for half in range(2):
            lo, hi = half * (JC // 2), (half + 1) * (JC // 2)
            nc.sync.dma_start(whb2_ch[:, lo:hi, :], cc_out_ch[:, lo:hi, :])
            nc.vector.tensor_copy(
                g2_ch[:, lo:hi, :], whb2_ch[:, lo:hi, D + 1: D + 2])

        # ---- phase 4: layer 2 ---- #
        pout2 = attention_unit(
            H, lhsT_of=lambda jc: whb2_sb[:, jc * AUG2: jc * AUG2 + AUG1],
            g_of=lambda jc: g2_sb[:, jc: jc + 1],
            f_row_dram=fgb2_d[0:1, :],
            n_aug=AUG1,
        )
        res2 = res.tile([D, R], F32, tag="res2")
        epilogue(pout2, res2[:], dst_f32=True)
        for ib in range(4):
            pt = psum.tile([128, D], F32, tag="bank")
            nc.tensor.transpose(
                pt[:], res2[:, ib * 128:(ib + 1) * 128], ident_sb[:])
            nc.vector.tensor_copy(out_sb[:, ib * D:(ib + 1) * D], pt[:])
        nc.sync.dma_start(
            out.ap().rearrange("(c p) w -> p c w", p=128),
            chunked_sb(out_sb[:], D))


# --------------------------------------------------------------------------- #
# host side
# --------------------------------------------------------------------------- #

def _pack_inputs(x, adj, W_heads, a_src, a_dst, W_out, a_src_out, a_dst_out):
    """Shard + repack the full inputs into the 8 per-core input maps."""
    x = np.asarray(x, np.float32)
    adj = np.asarray(adj)
    W_heads = np.asarray(W_heads, np.float32)
    a_src = np.asarray(a_src, np.float32)
    a_dst = np.asarray(a_dst, np.float32)
    W_out_np = np.asarray(W_out, np.float32)
    a_src_out = np.asarray(a_src_out, np.float32)
    a_dst_out = np.asarray(a_dst_out, np.float32)

    f16 = NPF16
    xT = np.ascontiguousarray(x.T).astype(f16)                       # [K, N]
    W_all = np.ascontiguousarray(
        W_heads.transpose(1, 0, 2).reshape(K, DALL)).astype(f16)     # [K, H*D]
    wa_cols = []
    for h in range(H):
        wa_cols.append(W_heads[h] @ a_src[h])
        wa_cols.append(W_heads[h] @ a_dst[h])
    wa = np.stack(wa_cols, axis=1).astype(f16)                       # [K, 16]
    W_out_p = W_out_np.astype(f16)                                   # [DALL, D]
    wa2 = np.stack([W_out_np @ a_src_out, W_out_np @ a_dst_out],
                   axis=1).astype(f16)                               # [DALL, 2]

    in_maps = []
    for c in range(NCORES):
        rows = slice(c * R, (c + 1) * R)
        adj_rows = (adj[rows, :] > 0).astype(np.float32)             # [R, N]
        adjB = np.ascontiguousarray(adj_rows.T).astype(f16)          # [N, R] 0/1
        in_maps.append({
            "xT": xT,
            "xrT": np.ascontiguousarray(x[rows].T).astype(f16),
            "adjB": adjB,
            "W_all": W_all,
            "wa": wa,
            "W_out": W_out_p,
            "wa2": wa2,
        })
    return in_maps


def kernel(**inputs) -> np.ndarray:
    if "nc" not in _CACHE:
        _CACHE["nc"] = _build(emulate_collective=False)
    nc = _CACHE["nc"]
    in_maps = _pack_inputs(**inputs)
    res = run_bass_kernel_spmd(nc, in_maps, core_ids=list(range(NCORES)))
    return np.concatenate([res.results[c]["out"] for c in range(NCORES)], axis=0)



# revision 2
# speedup vs baseline: 1.0828x; 1.0828x over previous
"""Trainium2 Bass kernel for a 2-layer dense GAT (nn_GAT_87144886436203).

Sharding: row-shard the N=4096 nodes across 8 NeuronCores (512 rows each).
Each core computes attention for its row block against all N columns, with the
contraction axis j on SBUF partitions so `att @ Wh` needs no transposes.

Score factorization: with s = f_i + g_j and alpha = 0.2,
    exp(leaky_relu(s)) = exp(alpha*s) * max(exp((1-alpha)*s), 1)
                       = [exp(alpha*f_i)] * exp(alpha*g_j) * max(G_i * H_j, 1)
where G = exp(0.8 f), H = exp(0.8 g).  The exp(alpha*f_i) factor is constant
per attention row and cancels in the softmax normalization, so it is dropped.
The whole [N, N] exp/leaky-relu work collapses to O(N) vector exps plus, per
[128, 512] score tile, ONE DVE tensor_scalar (4x perf mode)
    C~ = (G_rep * e^{g_j}) max e^{0.2 g_j}   (= e^{0.2 g_j} * max(G H, 1))
and one tensor_tensor mask multiply by the 0/1 adjacency per group.  The
row-sum (softmax denominator) rides a ones-column through the same matmul.

Engine balance knobs: S_GROUPS run the C-build on ACT instead of DVE as
    Cm1 = Prelu_0(H_j*F'_j * G_rep - F'_j') ... actually Relu(G*H - 1) with
    F'-prescaled weights Whb~ = e^{0.2 g_j} * [Wh | 1], which needs one extra
    correction matmul  pout += Whb~^T @ adj  per chunk (the "+1").
POOL_COUNT masks run on GPSIMD instead of DVE.

Wh is computed per-core for OWN rows only and all-gathered as
[Wh_h|1]x8 | f,g  (536 cols fp16) -- the same collective pattern as the
layer-2 gather -- instead of every core redoing the full [4096, 512] matmul.
"""

import numpy as np
import ml_dtypes

import concourse.bass as bass
import concourse.bacc as bacc
import concourse.tile as tile
import concourse.mybir as mybir
from concourse import masks
from concourse.bass_utils import run_bass_kernel_spmd

F16 = mybir.dt.float16
F32 = mybir.dt.float32
NPF16 = ml_dtypes.float16 if hasattr(ml_dtypes, "float16") else np.float16

NCORES = 8
N = 4096            # nodes
K = 512             # input feature dim (= NFEAT)
H = 8               # heads (layer 1)
D = 64              # per-head hidden (= NHID = NCLASS)
DALL = H * D        # 512
R = N // NCORES     # 512 rows per core
JC = N // 128       # 32 j-chunks
G = 4               # j-chunks per group (free dim 2048 for the mask tt)
NG = JC // G        # 8 groups
AUG1 = D + 1        # 65: [Wh_h | ones]
W1S = H * AUG1      # 520: [Wh_h|1]x8 per-chunk width
CW1 = W1S + 2 * H   # 536: gathered layer-1 row payload [Wh|1]x8 | f,g
CW2 = D + 2         # 66: gathered layer-2 payload [Wh2 | 1 | g2]
ALPHA = 0.2
N_UNITS = H + 1     # 8 heads + layer-2

# ---- engine-balance knobs ---------------------------------------------- #
S_GROUPS = (1, 3, 5, 7)   # group indices whose C-build runs on ACT
POOL_COUNT = 18           # of the 72 (unit, group) masks, run this many on Pool
NS = len(S_GROUPS)


def _bres(i, count, total):
    return (i * count) // total != ((i + 1) * count) // total


def _mask_on_pool(unit, g):
    return _bres(unit * NG + g, POOL_COUNT, N_UNITS * NG)


_CACHE = {}


# --------------------------------------------------------------------------- #
# device program
# --------------------------------------------------------------------------- #

def _build(emulate_collective=False):
    nc = bacc.Bacc(
        "TRN2",
        target_bir_lowering=False,
        debug=False,
        num_devices=1 if emulate_collective else NCORES,
    )

    xrT = nc.dram_tensor("xrT", [K, R], F16, kind="ExternalInput")
    adjB = nc.dram_tensor("adjB", [N, R], F16, kind="ExternalInput")
    W_all = nc.dram_tensor("W_all", [K, DALL], F16, kind="ExternalInput")
    wa = nc.dram_tensor("wa", [K, 2 * H], F16, kind="ExternalInput")
    W_out = nc.dram_tensor("W_out", [DALL, D], F16, kind="ExternalInput")
    wa2 = nc.dram_tensor("wa2", [DALL, 2], F16, kind="ExternalInput")
    out = nc.dram_tensor("out", [R, D], F32, kind="ExternalOutput")

    with tile.TileContext(nc) as tc:
        _emit(nc, tc, locals(), emulate_collective)

    nc.compile()
    return nc


def _emit(nc, tc, io, emulate_collective):
    xrT, adjB, W_all, wa, W_out, wa2, out = (
        io["xrT"], io["adjB"], io["W_all"], io["wa"],
        io["W_out"], io["wa2"], io["out"],
    )
    AT = mybir.AluOpType
    AF = mybir.ActivationFunctionType

    from contextlib import ExitStack
    with ExitStack() as ctx:
        res = ctx.enter_context(tc.tile_pool(name="res", bufs=1))
        psum = ctx.enter_context(tc.tile_pool(name="psum", bufs=4, space="PSUM"))
        ppool = ctx.enter_context(tc.tile_pool(name="ppool", bufs=4, space="PSUM"))
        work = ctx.enter_context(tc.tile_pool(name="work", bufs=3))
        work2 = ctx.enter_context(tc.tile_pool(name="work2", bufs=3))
        tpool = ctx.enter_context(tc.tile_pool(name="tpool", bufs=4))
        small = ctx.enter_context(tc.tile_pool(name="small", bufs=4))
        rpool = ctx.enter_context(tc.tile_pool(name="rpool", bufs=2))
        dram = ctx.enter_context(tc.tile_pool(name="dram", bufs=1, space="DRAM"))

        # ---- resident SBUF tensors ---- #
        xrT_sb = res.tile([128, 4 * R], F16, tag="xrT")
        adjB_sb = res.tile([128, JC * R], F16, tag="adjB")
        W_all_sb = res.tile([128, 4 * DALL], F16, tag="W_all")
        wa_sb = res.tile([128, 4 * 2 * H], F16, tag="wa")
        W_out_sb = res.tile([128, 4 * D], F16, tag="W_out")
        wa2_sb = res.tile([128, 4 * 2], F16, tag="wa2")
        whbig_sb = res.tile([128, JC * CW1], F16, tag="whbig")  # gathered L1
        whbt_sb = res.tile([128, NS * G * W1S], F16, tag="whbt")  # F'-scaled
        eg_sb = res.tile([128, JC * H], F32, tag="eg")     # exp(g)
        e02_sb = res.tile([128, JC * H], F32, tag="e02")   # exp(0.2 g)
        e08_sb = res.tile([128, JC * H], F32, tag="e08")   # exp(0.8 g)
        e02h_sb = res.tile([128, JC * H], F16, tag="e02h")
        hcatT_sb = res.tile([128, 4 * R], F16, tag="hcatT")
        whb2_sb = res.tile([128, JC * CW2], F16, tag="whb2")
        whb2t_sb = res.tile([128, NS * G * AUG1], F16, tag="whb2t")
        eg2_sb = res.tile([128, JC], F32, tag="eg2")
        e022_sb = res.tile([128, JC], F32, tag="e022")
        e082_sb = res.tile([128, JC], F32, tag="e082")
        ones_sb = res.tile([1, 128], F32, tag="ones")
        neg1_sb = res.tile([128, 1], F32, tag="neg1")
        ident_sb = res.tile([64, 64], F32, tag="ident")
        out_sb = res.tile([128, 4 * D], F32, tag="out_sb")

        def chunked(dram_t, width):
            return dram_t.ap().rearrange("(c p) w -> p c w", p=128)

        def chunked_sb(sb_ap, width):
            return sb_ap.rearrange("p (c w) -> p c w", w=width)

        def load(sb_tile, dram_t, width, split=1, split_free=1):
            dst = chunked_sb(sb_tile[:], width)
            src = chunked(dram_t, width)
            nch = dst.shape[1]
            step = max(1, nch // split)
            fstep = max(1, width // split_free)
            for lo in range(0, nch, step):
                hi = min(nch, lo + step)
                for flo in range(0, width, fstep):
                    fhi = min(width, flo + fstep)
                    nc.sync.dma_start(
                        dst[:, lo:hi, flo:fhi], src[:, lo:hi, flo:fhi])

        # ---- phase 0: loads + constants ---- #
        load(xrT_sb, xrT, R)
        load(wa_sb, wa, 2 * H)
        load(W_all_sb, W_all, DALL)
        load(adjB_sb, adjB, R, split=8)
        load(W_out_sb, W_out, D)
        load(wa2_sb, wa2, 2)
        nc.vector.memset(ones_sb[:], 1.0)
        nc.vector.memset(neg1_sb[:], -1.0)
        masks.make_identity(nc, ident_sb[:])

        whbig_ch = chunked_sb(whbig_sb[:], CW1)
        whb2_ch = chunked_sb(whb2_sb[:], CW2)

        # ---- phase A: own-row Wh/f/g + G-row, then allgather ---- #
        # G-row bounce: rows 2h hold exp(0.8 * f_h) for the frep broadcasts.
        pfr = psum.tile([16, R], F32, tag="bank")
        for kc in range(4):
            nc.tensor.matmul(
                pfr[:], wa_sb[:, kc * 2 * H:(kc + 1) * 2 * H],
                xrT_sb[:, kc * R:(kc + 1) * R],
                start=(kc == 0), stop=(kc == 3),
            )
        gx16 = res.tile([16, R], F16, tag="gx16")
        nc.scalar.activation(gx16[:], pfr[:], AF.Exp, scale=1.0 - ALPHA)
        fgb_d = dram.tile([16, R], F16, tag="fgb")
        nc.gpsimd.dma_start(fgb_d[:], gx16[:])

        gt1 = res.tile([128, 4 * CW1], F16, tag="gt1")
        nc.gpsimd.memset(gt1[:], 1.0)   # bakes the ones columns
        for ib in range(4):
            pw = psum.tile([128, DALL], F32, tag="bank")
            pf = psum.tile([128, 2 * H], F32, tag="bank")
            for kc in range(4):
                lhsT = xrT_sb[:, kc * R + ib * 128: kc * R + (ib + 1) * 128]
                nc.tensor.matmul(
                    pw[:], lhsT, W_all_sb[:, kc * DALL:(kc + 1) * DALL],
                    start=(kc == 0), stop=(kc == 3))
                nc.tensor.matmul(
                    pf[:], lhsT, wa_sb[:, kc * 2 * H:(kc + 1) * 2 * H],
                    start=(kc == 0), stop=(kc == 3))
            dst = gt1[:, ib * CW1: ib * CW1 + W1S].rearrange(
                "p (h x) -> p h x", x=AUG1)[:, :, 0:D]
            nc.scalar.activation(
                dst, pw.rearrange("p (h x) -> p h x", x=D), AF.Copy)
            nc.vector.tensor_copy(
                gt1[:, ib * CW1 + W1S:(ib + 1) * CW1], pf[:])

        cc1_in = dram.tile([R, CW1], F16, tag="cc1_in")
        cc_space = {} if emulate_collective else {"addr_space": "Shared"}
        cc1_out = dram.tile([N, CW1], F16, tag="cc1_out", **cc_space)
        nc.sync.dma_start(
            cc1_in[:].rearrange("(c p) w -> p c w", p=128),
            chunked_sb(gt1[:], CW1))
        if emulate_collective:
            for c in range(NCORES):
                nc.sync.dma_start(cc1_out[c * R:(c + 1) * R, :], cc1_in[:])
        else:
            nc.gpsimd.collective_compute(
                "AllGather", mybir.AluOpType.bypass,
                replica_groups=[list(range(NCORES))],
                ins=[cc1_in.opt()], outs=[cc1_out.opt()],
            )

        # ---- phase B: land the gather + per-node exp vectors ---- #
        cc1_out_ch = cc1_out[:].rearrange("(c p) w -> p c w", p=128)
        for half in range(2):
            lo, hi = half * (JC // 2), (half + 1) * (JC // 2)
            nc.sync.dma_start(whbig_ch[:, lo:hi, :], cc1_out_ch[:, lo:hi, :])
            gcols = whbig_ch[:, lo:hi, W1S:CW1].rearrange(
                "p c (h two) -> p c h two", two=2)[:, :, :, 1:2]
            for e_sb, sc in ((eg_sb, 1.0), (e02_sb, ALPHA), (e08_sb, 1.0 - ALPHA)):
                dst = e_sb[:, lo * H:hi * H].rearrange(
                    "p (c h) -> p c h", h=H).unsqueeze(3)
                nc.scalar.activation(dst, gcols, AF.Exp, scale=sc)
            nc.vector.tensor_copy(
                e02h_sb[:, lo * H:hi * H], e02_sb[:, lo * H:hi * H])

        # F'-scaled weights for the ACT-built chunks (all 8 heads at once)
        for si, g in enumerate(S_GROUPS):
            for c in range(G):
                jc = g * G + c
                k = si * G + c
                src = whbig_ch[:, jc, 0:W1S].rearrange(
                    "p (h x) -> p h x", x=AUG1)
                fb = e02h_sb[:, jc * H:(jc + 1) * H].unsqueeze(2) \
                    .broadcast_to([128, H, AUG1])
                nc.gpsimd.tensor_tensor(
                    whbt_sb[:, k * W1S:(k + 1) * W1S].rearrange(
                        "p (h x) -> p h x", x=AUG1),
                    src, fb, AT.mult)

        # ---- attention unit ---- #
        def unit_start(f_row_dram):
            pout = ppool.tile([AUG1, R], F32, tag="pout")
            frep = tpool.tile([128, R], F16, tag="frep")
            nc.gpsimd.dma_start(frep[:], f_row_dram.broadcast_to([128, R]))
            return pout, frep

        def unit_group(unit, pout, frep, g, mm, lhsT_of, lhsTs_of,
                       eg_of, e02_of, e08_of):
            """mm = [next_idx, total]; returns updated mm."""
            on_act = g in S_GROUPS
            si = S_GROUPS.index(g) if on_act else None
            u = work.tile([128, G * R], F16, tag="u")
            for c in range(G):
                jc = g * G + c
                if on_act:
                    nc.scalar.activation(
                        u[:, c * R:(c + 1) * R], frep[:],
                        AF.Prelu, bias=neg1_sb[:], scale=e08_of(jc),
                        alpha=0.0)
                else:
                    nc.vector.tensor_scalar(
                        u[:, c * R:(c + 1) * R], frep[:],
                        eg_of(jc), e02_of(jc), AT.mult, AT.max)
            pm = work2.tile([128, G * R], F16, tag="pm")
            eng = nc.gpsimd if _mask_on_pool(unit, g) else nc.vector
            eng.tensor_tensor(
                pm[:], u[:], adjB_sb[:, g * G * R:(g + 1) * G * R], AT.mult)
            for c in range(G):
                jc = g * G + c
                lhsT = lhsTs_of(si * G + c) if on_act else lhsT_of(jc)
                nc.tensor.matmul(
                    pout[:], lhsT, pm[:, c * R:(c + 1) * R],
                    start=(mm[0] == 0), stop=(mm[0] == mm[1] - 1))
                mm[0] += 1
            if on_act:
                for c in range(G):
                    jc = g * G + c
                    nc.tensor.matmul(
                        pout[:], lhsTs_of(si * G + c),
                        adjB_sb[:, jc * R:(jc + 1) * R],
                        start=(mm[0] == 0), stop=(mm[0] == mm[1] - 1))
                    mm[0] += 1

        def epilogue(pout, dst_ap, dst_f32):
            """dst = elu(att_out / rowsum) written to dst_ap ([64, R])."""
            dt = F32 if dst_f32 else F16
            recip = rpool.tile([1, R], F32, tag="recip")
            nc.vector.reciprocal(recip[:], pout[D:D + 1, :])
            pr = psum.tile([D, R], F32, tag="bank")
            nc.tensor.matmul(pr[:], ones_sb[0:1, 0:D], recip[:])
            rsb = small.tile([D, R], F32, tag="ep")
            nc.any.tensor_copy(rsb[:], pr[:])
            hl = small.tile([D, R], dt, tag="ep")
            nc.vector.tensor_tensor(hl[:], pout[0:D, :], rsb[:], AT.mult)
            # elu(x) = max(x,0) + min(exp(x),1) - 1   (exp monotone)
            q = small.tile([D, R], dt, tag="ep")
            nc.scalar.activation(q[:], hl[:], AF.Exp)
            t1 = small.tile([D, R], dt, tag="ep")
            nc.vector.tensor_scalar(t1[:], q[:], 1.0, -1.0, AT.min, AT.add)
            t2 = small.tile([D, R], dt, tag="ep")
            nc.vector.tensor_scalar(t2[:], hl[:], 0.0, None, AT.max)
            nc.vector.tensor_tensor(dst_ap, t1[:], t2[:], AT.add)

        # ---- phase C: layer-1 heads, two at a time ---- #
        MM_TOTAL = JC + NS * G

        def l1_args(h):
            return (
                lambda jc, h=h: whbig_ch[:, jc, h * AUG1:(h + 1) * AUG1],
                lambda k, h=h: whbt_sb[:, k * W1S + h * AUG1:
                                       k * W1S + (h + 1) * AUG1],
                lambda jc, h=h: eg_sb[:, jc * H + h: jc * H + h + 1],
                lambda jc, h=h: e02_sb[:, jc * H + h: jc * H + h + 1],
                lambda jc, h=h: e08_sb[:, jc * H + h: jc * H + h + 1],
            )

        for hp in range(0, H, 2):
            pair = []
            for h in (hp, hp + 1):
                pout, frep = unit_start(fgb_d[2 * h:2 * h + 1, :])
                pair.append([h, pout, frep, [0, MM_TOTAL], l1_args(h)])
            for g in range(NG):
                for (h, pout, frep, mm, args) in pair:
                    unit_group(h, pout, frep, g, mm, *args)
            for (h, pout, frep, mm, args) in pair:
                kc, po = h // 2, (h % 2) * D
                epilogue(pout, hcatT_sb[po:po + D, kc * R:(kc + 1) * R],
                         dst_f32=False)

        # ---- phase D: layer-2 prep + allgather ---- #
        gt2 = res.tile([128, 4 * CW2], F16, tag="gt2")
        nc.vector.memset(gt2[:], 1.0)
        for ib in range(4):
            pw2 = psum.tile([128, D], F32, tag="bank")
            pg2 = psum.tile([128, 2], F32, tag="bank")
            for kc in range(4):
                lhsT = hcatT_sb[:, kc * R + ib * 128: kc * R + (ib + 1) * 128]
                nc.tensor.matmul(pw2[:], lhsT, W_out_sb[:, kc * D:(kc + 1) * D],
                                 start=(kc == 0), stop=(kc == 3))
                nc.tensor.matmul(pg2[:], lhsT, wa2_sb[:, kc * 2:(kc + 1) * 2],
                                 start=(kc == 0), stop=(kc == 3))
            nc.vector.tensor_copy(gt2[:, ib * CW2: ib * CW2 + D], pw2[:])
            nc.vector.tensor_copy(
                gt2[:, ib * CW2 + D + 1: ib * CW2 + D + 2], pg2[:, 1:2])

        pfg2 = psum.tile([2, R], F32, tag="bank")
        for kc in range(4):
            nc.tensor.matmul(pfg2[:], wa2_sb[:, kc * 2:(kc + 1) * 2],
                             hcatT_sb[:, kc * R:(kc + 1) * R],
                             start=(kc == 0), stop=(kc == 3))
        g2row = res.tile([1, R], F16, tag="g2row")
        nc.scalar.activation(g2row[:], pfg2[0:1, :], AF.Exp, scale=1.0 - ALPHA)
        fgb2_d = dram.tile([1, R], F16, tag="fgb2")
        nc.sync.dma_start(fgb2_d[:], g2row[:])

        cc2_in = dram.tile([R, CW2], F16, tag="cc2_in")
        cc2_out = dram.tile([N, CW2], F16, tag="cc2_out", **cc_space)
        nc.sync.dma_start(
            cc2_in[:].rearrange("(c p) w -> p c w", p=128),
            chunked_sb(gt2[:], CW2))
        if emulate_collective:
            for c in range(NCORES):
                nc.sync.dma_start(cc2_out[c * R:(c + 1) * R, :], cc2_in[:])
        else:
            nc.gpsimd.collective_compute(
                "AllGather", mybir.AluOpType.bypass,
                replica_groups=[list(range(NCORES))],
                ins=[cc2_in.opt()], outs=[cc2_out.opt()],
            )
        cc2_out_ch = cc2_out[:].rearrange("(c p) w -> p c w", p=128)
        for half in range(2):
            lo, hi = half * (JC // 2), (half + 1) * (JC // 2)
            nc.sync.dma_start(whb2_ch[:, lo:hi, :], cc2_out_ch[:, lo:hi, :])
            gcols2 = whb2_ch[:, lo:hi, D + 1:D + 2]
            for e_sb, sc in ((eg2_sb, 1.0), (e022_sb, ALPHA),
                             (e082_sb, 1.0 - ALPHA)):
                nc.scalar.activation(
                    e_sb[:, lo:hi].unsqueeze(2), gcols2, AF.Exp, scale=sc)
        for si, g in enumerate(S_GROUPS):
            for c in range(G):
                jc = g * G + c
                k = si * G + c
                nc.vector.tensor_scalar(
                    whb2t_sb[:, k * AUG1:(k + 1) * AUG1],
                    whb2_ch[:, jc, 0:AUG1],
                    e022_sb[:, jc:jc + 1], None, AT.mult)

        # ---- phase E: layer 2 ---- #
        pout2, frep2 = unit_start(fgb2_d[0:1, :])
        mm2 = [0, MM_TOTAL]
        args2 = (
            lambda jc: whb2_ch[:, jc, 0:AUG1],
            lambda k: whb2t_sb[:, k * AUG1:(k + 1) * AUG1],
            lambda jc: eg2_sb[:, jc:jc + 1],
            lambda jc: e022_sb[:, jc:jc + 1],
            lambda jc: e082_sb[:, jc:jc + 1],
        )
        for g in range(NG):
            unit_group(H, pout2, frep2, g, mm2, *args2)
        res2 = res.tile([D, R], F32, tag="res2")
        epilogue(pout2, res2[:], dst_f32=True)
        for ib in range(4):
            pt = psum.tile([128, D], F32, tag="bank")
            nc.tensor.transpose(
                pt[:], res2[:, ib * 128:(ib + 1) * 128], ident_sb[:])
            nc.vector.tensor_copy(out_sb[:, ib * D:(ib + 1) * D], pt[:])
        nc.sync.dma_start(
            out.ap().rearrange("(c p) w -> p c w", p=128),
            chunked_sb(out_sb[:], D))


# --------------------------------------------------------------------------- #
# host side
# --------------------------------------------------------------------------- #

def _pack_inputs(x, adj, W_heads, a_src, a_dst, W_out, a_src_out, a_dst_out):
    """Shard + repack the full inputs into the 8 per-core input maps."""
    x = np.asarray(x, np.float32)
    adj = np.asarray(adj)
    W_heads = np.asarray(W_heads, np.float32)
    a_src = np.asarray(a_src, np.float32)
    a_dst = np.asarray(a_dst, np.float32)
    W_out_np = np.asarray(W_out, np.float32)
    a_src_out = np.asarray(a_src_out, np.float32)
    a_dst_out = np.asarray(a_dst_out, np.float32)

    f16 = NPF16
    W_all = np.ascontiguousarray(
        W_heads.transpose(1, 0, 2).reshape(K, DALL)).astype(f16)     # [K, H*D]
    wa_cols = []
    for h in range(H):
        wa_cols.append(W_heads[h] @ a_src[h])
        wa_cols.append(W_heads[h] @ a_dst[h])
    wa = np.stack(wa_cols, axis=1).astype(f16)                       # [K, 16]
    W_out_p = W_out_np.astype(f16)                                   # [DALL, D]
    wa2 = np.stack([W_out_np @ a_src_out, W_out_np @ a_dst_out],
                   axis=1).astype(f16)                               # [DALL, 2]

    in_maps = []
    for c in range(NCORES):
        rows = slice(c * R, (c + 1) * R)
        adj_rows = (adj[rows, :] > 0).astype(np.float32)             # [R, N]
        adjB = np.ascontiguousarray(adj_rows.T).astype(f16)          # [N, R] 0/1
        in_maps.append({
            "xrT": np.ascontiguousarray(x[rows].T).astype(f16),
            "adjB": adjB,
            "W_all": W_all,
            "wa": wa,
            "W_out": W_out_p,
            "wa2": wa2,
        })
    return in_maps


def kernel(**inputs) -> np.ndarray:
    if "nc" not in _CACHE:
        _CACHE["nc"] = _build(emulate_collective=False)
    nc = _CACHE["nc"]
    in_maps = _pack_inputs(**inputs)
    res = run_bass_kernel_spmd(nc, in_maps, core_ids=list(range(NCORES)))
    return np.concatenate([res.results[c]["out"] for c in range(NCORES)], axis=0)


# revision 3
# speedup vs baseline: 1.1595x; 1.0709x over previous
"""Trainium2 Bass kernel for a 2-layer dense GAT (nn_GAT_87144886436203).

Sharding: row-shard the N=4096 nodes across 8 NeuronCores (512 rows each).
Each core computes attention for its row block against all N columns, with the
contraction axis j on SBUF partitions so `att @ Wh` needs no transposes.

Score factorization: with s = f_i + g_j and alpha = 0.2,
    exp(leaky_relu(s)) = exp(alpha*s) * max(exp((1-alpha)*s), 1)
                       = [exp(alpha*f_i)] * exp(alpha*g_j) * max(G_i * H_j, 1)
where G = exp(0.8 f), H = exp(0.8 g).  The exp(alpha*f_i) factor is constant
per attention row and cancels in the softmax normalization, so it is dropped.
The whole [N, N] exp/leaky-relu work collapses to O(N) vector exps plus, per
[128, 512] score tile, ONE DVE tensor_scalar (4x perf mode)
    C~ = (G_rep * e^{g_j}) max e^{0.2 g_j}   (= e^{0.2 g_j} * max(G H, 1))
and one tensor_tensor mask multiply by the 0/1 adjacency per group.  The
row-sum (softmax denominator) rides a ones-column through the same matmul.

Engine balance knobs: S_GROUPS run the C-build on ACT instead of DVE as
    Cm1 = Prelu_0(H_j*F'_j * G_rep - F'_j') ... actually Relu(G*H - 1) with
    F'-prescaled weights Whb~ = e^{0.2 g_j} * [Wh | 1], which needs one extra
    correction matmul  pout += Whb~^T @ adj  per chunk (the "+1").
POOL_COUNT masks run on GPSIMD instead of DVE.

Wh is computed per-core for OWN rows only and all-gathered as
[Wh_h|1]x8 | f,g  (536 cols fp16) -- the same collective pattern as the
layer-2 gather -- instead of every core redoing the full [4096, 512] matmul.
"""

import numpy as np
import ml_dtypes

import concourse.bass as bass
import concourse.bacc as bacc
import concourse.tile as tile
import concourse.mybir as mybir
from concourse import masks
from concourse.bass_utils import run_bass_kernel_spmd

F16 = mybir.dt.float16
F32 = mybir.dt.float32
NPF16 = ml_dtypes.float16 if hasattr(ml_dtypes, "float16") else np.float16

NCORES = 8
N = 4096            # nodes
K = 512             # input feature dim (= NFEAT)
H = 8               # heads (layer 1)
D = 64              # per-head hidden (= NHID = NCLASS)
DALL = H * D        # 512
R = N // NCORES     # 512 rows per core
JC = N // 128       # 32 j-chunks
G = 4               # j-chunks per group (free dim 2048 for the mask tt)
NG = JC // G        # 8 groups
AUG1 = D + 1        # 65: [Wh_h | ones]
W1S = H * AUG1      # 520: [Wh_h|1]x8 per-chunk width
CW1 = W1S + 2 * H   # 536: gathered layer-1 row payload [Wh|1]x8 | f,g
CW2 = D + 2         # 66: gathered layer-2 payload [Wh2 | 1 | g2]
ALPHA = 0.2
N_UNITS = H + 1     # 8 heads + layer-2

# ---- engine-balance knobs ---------------------------------------------- #
S_GROUPS = (1, 3, 5, 7)   # group indices whose C-build runs on ACT
POOL_COUNT = 18           # of the 72 (unit, group) masks, run this many on Pool
NS = len(S_GROUPS)


def _bres(i, count, total):
    return (i * count) // total != ((i + 1) * count) // total


def _mask_on_pool(unit, g):
    return _bres(unit * NG + g, POOL_COUNT, N_UNITS * NG)


_CACHE = {}


# --------------------------------------------------------------------------- #
# device program
# --------------------------------------------------------------------------- #

def _build(emulate_collective=False):
    nc = bacc.Bacc(
        "TRN2",
        target_bir_lowering=False,
        debug=False,
        num_devices=1 if emulate_collective else NCORES,
    )

    xrT = nc.dram_tensor("xrT", [K, R], F16, kind="ExternalInput")
    adjB = nc.dram_tensor("adjB", [N, R], F16, kind="ExternalInput")
    W_all = nc.dram_tensor("W_all", [K, DALL], F16, kind="ExternalInput")
    wa = nc.dram_tensor("wa", [K, 2 * H], F16, kind="ExternalInput")
    W_out = nc.dram_tensor("W_out", [DALL, D], F16, kind="ExternalInput")
    wa2 = nc.dram_tensor("wa2", [DALL, 2], F16, kind="ExternalInput")
    out = nc.dram_tensor("out", [R, D], F32, kind="ExternalOutput")

    with tile.TileContext(nc) as tc:
        _emit(nc, tc, locals(), emulate_collective)

    nc.compile()
    return nc


def _emit(nc, tc, io, emulate_collective):
    xrT, adjB, W_all, wa, W_out, wa2, out = (
        io["xrT"], io["adjB"], io["W_all"], io["wa"],
        io["W_out"], io["wa2"], io["out"],
    )
    AT = mybir.AluOpType
    AF = mybir.ActivationFunctionType

    from contextlib import ExitStack
    with ExitStack() as ctx:
        res = ctx.enter_context(tc.tile_pool(name="res", bufs=1))
        psum = ctx.enter_context(tc.tile_pool(name="psum", bufs=4, space="PSUM"))
        ppool = ctx.enter_context(tc.tile_pool(name="ppool", bufs=4, space="PSUM"))
        work = ctx.enter_context(tc.tile_pool(name="work", bufs=3))
        work2 = ctx.enter_context(tc.tile_pool(name="work2", bufs=3))
        tpool = ctx.enter_context(tc.tile_pool(name="tpool", bufs=4))
        small = ctx.enter_context(tc.tile_pool(name="small", bufs=4))
        rpool = ctx.enter_context(tc.tile_pool(name="rpool", bufs=2))
        dram = ctx.enter_context(tc.tile_pool(name="dram", bufs=1, space="DRAM"))

        # ---- resident SBUF tensors ---- #
        xrT_sb = res.tile([128, 4 * R], F16, tag="xrT")
        adjB_sb = res.tile([128, JC * R], F16, tag="adjB")
        W_all_sb = res.tile([128, 4 * DALL], F16, tag="W_all")
        wa_sb = res.tile([128, 4 * 2 * H], F16, tag="wa")
        W_out_sb = res.tile([128, 4 * D], F16, tag="W_out")
        wa2_sb = res.tile([128, 4 * 2], F16, tag="wa2")
        whbig_sb = res.tile([128, JC * CW1], F16, tag="whbig")  # gathered L1
        whbt_sb = res.tile([128, NS * G * W1S], F16, tag="whbt")  # F'-scaled
        eg_sb = res.tile([128, JC * H], F32, tag="eg")     # exp(g)
        e02_sb = res.tile([128, JC * H], F32, tag="e02")   # exp(0.2 g)
        e08_sb = res.tile([128, JC * H], F32, tag="e08")   # exp(0.8 g)
        e02h_sb = res.tile([128, JC * H], F16, tag="e02h")
        hcatT_sb = res.tile([128, 4 * R], F16, tag="hcatT")
        whb2_sb = res.tile([128, JC * CW2], F16, tag="whb2")
        whb2t_sb = res.tile([128, NS * G * AUG1], F16, tag="whb2t")
        eg2_sb = res.tile([128, JC], F32, tag="eg2")
        e022_sb = res.tile([128, JC], F32, tag="e022")
        e082_sb = res.tile([128, JC], F32, tag="e082")
        ones_sb = res.tile([1, 128], F32, tag="ones")
        neg1_sb = res.tile([128, 1], F32, tag="neg1")
        ident_sb = res.tile([64, 64], F32, tag="ident")
        out_sb = res.tile([128, 4 * D], F32, tag="out_sb")

        def chunked(dram_t, width):
            return dram_t.ap().rearrange("(c p) w -> p c w", p=128)

        def chunked_sb(sb_ap, width):
            return sb_ap.rearrange("p (c w) -> p c w", w=width)

        def load(sb_tile, dram_t, width, split=1, split_free=1):
            dst = chunked_sb(sb_tile[:], width)
            src = chunked(dram_t, width)
            nch = dst.shape[1]
            step = max(1, nch // split)
            fstep = max(1, width // split_free)
            for lo in range(0, nch, step):
                hi = min(nch, lo + step)
                for flo in range(0, width, fstep):
                    fhi = min(width, flo + fstep)
                    nc.sync.dma_start(
                        dst[:, lo:hi, flo:fhi], src[:, lo:hi, flo:fhi])

        # ---- phase 0: loads + constants ---- #
        load(xrT_sb, xrT, R)
        load(wa_sb, wa, 2 * H)
        load(W_all_sb, W_all, DALL)
        load(adjB_sb, adjB, R, split=8)
        load(W_out_sb, W_out, D)
        load(wa2_sb, wa2, 2)
        nc.vector.memset(ones_sb[:], 1.0)
        nc.vector.memset(neg1_sb[:], -1.0)
        masks.make_identity(nc, ident_sb[:])

        whbig_ch = chunked_sb(whbig_sb[:], CW1)
        whb2_ch = chunked_sb(whb2_sb[:], CW2)

        # ---- phase A: own-row Wh/f/g + G-row, then allgather ---- #
        # G-row bounce: rows 2h hold exp(0.8 * f_h) for the frep broadcasts.
        pfr = psum.tile([16, R], F32, tag="bank")
        for kc in range(4):
            nc.tensor.matmul(
                pfr[:], wa_sb[:, kc * 2 * H:(kc + 1) * 2 * H],
                xrT_sb[:, kc * R:(kc + 1) * R],
                start=(kc == 0), stop=(kc == 3),
            )
        gx16 = res.tile([16, R], F16, tag="gx16")
        nc.scalar.activation(gx16[:], pfr[:], AF.Exp, scale=1.0 - ALPHA)
        fgb_d = dram.tile([16, R], F16, tag="fgb")
        nc.gpsimd.dma_start(fgb_d[:], gx16[:])

        gt1 = res.tile([128, 4 * CW1], F16, tag="gt1")
        nc.gpsimd.memset(gt1[:], 1.0)   # bakes the ones columns
        for ib in range(4):
            pw = psum.tile([128, DALL], F32, tag="bank")
            pf = psum.tile([128, 2 * H], F32, tag="bank")
            for kc in range(4):
                lhsT = xrT_sb[:, kc * R + ib * 128: kc * R + (ib + 1) * 128]
                nc.tensor.matmul(
                    pw[:], lhsT, W_all_sb[:, kc * DALL:(kc + 1) * DALL],
                    start=(kc == 0), stop=(kc == 3))
                nc.tensor.matmul(
                    pf[:], lhsT, wa_sb[:, kc * 2 * H:(kc + 1) * 2 * H],
                    start=(kc == 0), stop=(kc == 3))
            dst = gt1[:, ib * CW1: ib * CW1 + W1S].rearrange(
                "p (h x) -> p h x", x=AUG1)[:, :, 0:D]
            nc.scalar.activation(
                dst, pw.rearrange("p (h x) -> p h x", x=D), AF.Copy)
            nc.vector.tensor_copy(
                gt1[:, ib * CW1 + W1S:(ib + 1) * CW1], pf[:])

        cc1_in = dram.tile([R, CW1], F16, tag="cc1_in")
        cc_space = {} if emulate_collective else {"addr_space": "Shared"}
        cc1_out = dram.tile([N, CW1], F16, tag="cc1_out", **cc_space)
        nc.sync.dma_start(
            cc1_in[:].rearrange("(c p) w -> p c w", p=128),
            chunked_sb(gt1[:], CW1))
        if emulate_collective:
            for c in range(NCORES):
                nc.sync.dma_start(cc1_out[c * R:(c + 1) * R, :], cc1_in[:])
        else:
            nc.gpsimd.collective_compute(
                "AllGather", mybir.AluOpType.bypass,
                replica_groups=[list(range(NCORES))],
                ins=[cc1_in.opt()], outs=[cc1_out.opt()],
            )

        # ---- phase B: land the gather + per-node exp vectors ---- #
        cc1_out_ch = cc1_out[:].rearrange("(c p) w -> p c w", p=128)
        for half in range(2):
            lo, hi = half * (JC // 2), (half + 1) * (JC // 2)
            nc.sync.dma_start(whbig_ch[:, lo:hi, :], cc1_out_ch[:, lo:hi, :])
            gcols = whbig_ch[:, lo:hi, W1S:CW1].rearrange(
                "p c (h two) -> p c h two", two=2)[:, :, :, 1:2]
            for e_sb, sc in ((eg_sb, 1.0), (e02_sb, ALPHA), (e08_sb, 1.0 - ALPHA)):
                dst = e_sb[:, lo * H:hi * H].rearrange(
                    "p (c h) -> p c h", h=H).unsqueeze(3)
                nc.scalar.activation(dst, gcols, AF.Exp, scale=sc)
            nc.vector.tensor_copy(
                e02h_sb[:, lo * H:hi * H], e02_sb[:, lo * H:hi * H])

        # F'-scaled weights for the ACT-built chunks (all 8 heads at once)
        for si, g in enumerate(S_GROUPS):
            for c in range(G):
                jc = g * G + c
                k = si * G + c
                src = whbig_ch[:, jc, 0:W1S].rearrange(
                    "p (h x) -> p h x", x=AUG1)
                fb = e02h_sb[:, jc * H:(jc + 1) * H].unsqueeze(2) \
                    .broadcast_to([128, H, AUG1])
                nc.gpsimd.tensor_tensor(
                    whbt_sb[:, k * W1S:(k + 1) * W1S].rearrange(
                        "p (h x) -> p h x", x=AUG1),
                    src, fb, AT.mult)

        # ---- attention unit ---- #
        def unit_start(f_row_dram):
            pout = ppool.tile([AUG1, R], F32, tag="pout")
            frep = tpool.tile([128, R], F16, tag="frep")
            nc.gpsimd.dma_start(frep[:], f_row_dram.broadcast_to([128, R]))
            return pout, frep

        def unit_group(unit, pout, frep, g, mm, lhsT_of, lhsTs_of,
                       eg_of, e02_of, e08_of):
            """mm = [next_idx, total]; returns updated mm."""
            on_act = g in S_GROUPS
            si = S_GROUPS.index(g) if on_act else None
            u = work.tile([128, G * R], F16, tag="u")
            for c in range(G):
                jc = g * G + c
                if on_act:
                    nc.scalar.activation(
                        u[:, c * R:(c + 1) * R], frep[:],
                        AF.Prelu, bias=neg1_sb[:], scale=e08_of(jc),
                        alpha=0.0)
                else:
                    nc.vector.tensor_scalar(
                        u[:, c * R:(c + 1) * R], frep[:],
                        eg_of(jc), e02_of(jc), AT.mult, AT.max)
            pm = work2.tile([128, G * R], F16, tag="pm")
            eng = nc.gpsimd if _mask_on_pool(unit, g) else nc.vector
            eng.tensor_tensor(
                pm[:], u[:], adjB_sb[:, g * G * R:(g + 1) * G * R], AT.mult)
            for c in range(G):
                jc = g * G + c
                lhsT = lhsTs_of(si * G + c) if on_act else lhsT_of(jc)
                nc.tensor.matmul(
                    pout[:], lhsT, pm[:, c * R:(c + 1) * R],
                    start=(mm[0] == 0), stop=(mm[0] == mm[1] - 1))
                mm[0] += 1
            if on_act:
                for c in range(G):
                    jc = g * G + c
                    nc.tensor.matmul(
                        pout[:], lhsTs_of(si * G + c),
                        adjB_sb[:, jc * R:(jc + 1) * R],
                        start=(mm[0] == 0), stop=(mm[0] == mm[1] - 1))
                    mm[0] += 1

        def epilogue(pout, dst_ap, dst_f32):
            """dst = elu(att_out / rowsum) written to dst_ap ([64, R])."""
            dt = F32 if dst_f32 else F16
            recip = rpool.tile([1, R], F32, tag="recip")
            nc.vector.reciprocal(recip[:], pout[D:D + 1, :])
            pr = psum.tile([D, R], F32, tag="bank")
            nc.tensor.matmul(pr[:], ones_sb[0:1, 0:D], recip[:])
            rsb = small.tile([D, R], F32, tag="ep")
            nc.any.tensor_copy(rsb[:], pr[:])
            hl = small.tile([D, R], dt, tag="ep")
            nc.vector.tensor_tensor(hl[:], pout[0:D, :], rsb[:], AT.mult)
            # elu(x) = max(x,0) + min(exp(x),1) - 1   (exp monotone)
            q = small.tile([D, R], dt, tag="ep")
            nc.scalar.activation(q[:], hl[:], AF.Exp)
            t1 = small.tile([D, R], dt, tag="ep")
            nc.vector.tensor_scalar(t1[:], q[:], 1.0, -1.0, AT.min, AT.add)
            t2 = small.tile([D, R], dt, tag="ep")
            nc.vector.tensor_scalar(t2[:], hl[:], 0.0, None, AT.max)
            nc.vector.tensor_tensor(dst_ap, t1[:], t2[:], AT.add)

        # ---- phase C: layer-1 heads, two at a time ---- #
        MM_TOTAL = JC + NS * G

        def l1_args(h):
            return (
                lambda jc, h=h: whbig_ch[:, jc, h * AUG1:(h + 1) * AUG1],
                lambda k, h=h: whbt_sb[:, k * W1S + h * AUG1:
                                       k * W1S + (h + 1) * AUG1],
                lambda jc, h=h: eg_sb[:, jc * H + h: jc * H + h + 1],
                lambda jc, h=h: e02_sb[:, jc * H + h: jc * H + h + 1],
                lambda jc, h=h: e08_sb[:, jc * H + h: jc * H + h + 1],
            )

        for hp in range(0, H, 2):
            pair = []
            for h in (hp, hp + 1):
                pout, frep = unit_start(fgb_d[2 * h:2 * h + 1, :])
                pair.append([h, pout, frep, [0, MM_TOTAL], l1_args(h)])
            for gi in range(NG):
                for pi, (h, pout, frep, mm, args) in enumerate(pair):
                    # stagger the pair by one group so one unit is in an
                    # ACT-built group while the other is in a DVE-built one
                    unit_group(h, pout, frep, (gi + pi) % NG, mm, *args)
            for (h, pout, frep, mm, args) in pair:
                kc, po = h // 2, (h % 2) * D
                epilogue(pout, hcatT_sb[po:po + D, kc * R:(kc + 1) * R],
                         dst_f32=False)

        # ---- phase D: layer-2 prep + allgather ---- #
        gt2 = res.tile([128, 4 * CW2], F16, tag="gt2")
        nc.vector.memset(gt2[:], 1.0)
        for ib in range(4):
            pw2 = psum.tile([128, D], F32, tag="bank")
            pg2 = psum.tile([128, 2], F32, tag="bank")
            for kc in range(4):
                lhsT = hcatT_sb[:, kc * R + ib * 128: kc * R + (ib + 1) * 128]
                nc.tensor.matmul(pw2[:], lhsT, W_out_sb[:, kc * D:(kc + 1) * D],
                                 start=(kc == 0), stop=(kc == 3))
                nc.tensor.matmul(pg2[:], lhsT, wa2_sb[:, kc * 2:(kc + 1) * 2],
                                 start=(kc == 0), stop=(kc == 3))
            nc.vector.tensor_copy(gt2[:, ib * CW2: ib * CW2 + D], pw2[:])
            nc.vector.tensor_copy(
                gt2[:, ib * CW2 + D + 1: ib * CW2 + D + 2], pg2[:, 1:2])

        pfg2 = psum.tile([2, R], F32, tag="bank")
        for kc in range(4):
            nc.tensor.matmul(pfg2[:], wa2_sb[:, kc * 2:(kc + 1) * 2],
                             hcatT_sb[:, kc * R:(kc + 1) * R],
                             start=(kc == 0), stop=(kc == 3))
        g2row = res.tile([1, R], F16, tag="g2row")
        nc.scalar.activation(g2row[:], pfg2[0:1, :], AF.Exp, scale=1.0 - ALPHA)
        fgb2_d = dram.tile([1, R], F16, tag="fgb2")
        nc.sync.dma_start(fgb2_d[:], g2row[:])

        cc2_in = dram.tile([R, CW2], F16, tag="cc2_in")
        cc2_out = dram.tile([N, CW2], F16, tag="cc2_out", **cc_space)
        nc.sync.dma_start(
            cc2_in[:].rearrange("(c p) w -> p c w", p=128),
            chunked_sb(gt2[:], CW2))
        if emulate_collective:
            for c in range(NCORES):
                nc.sync.dma_start(cc2_out[c * R:(c + 1) * R, :], cc2_in[:])
        else:
            nc.gpsimd.collective_compute(
                "AllGather", mybir.AluOpType.bypass,
                replica_groups=[list(range(NCORES))],
                ins=[cc2_in.opt()], outs=[cc2_out.opt()],
            )
        cc2_out_ch = cc2_out[:].rearrange("(c p) w -> p c w", p=128)
        for half in range(2):
            lo, hi = half * (JC // 2), (half + 1) * (JC // 2)
            nc.sync.dma_start(whb2_ch[:, lo:hi, :], cc2_out_ch[:, lo:hi, :])
            gcols2 = whb2_ch[:, lo:hi, D + 1:D + 2]
            for e_sb, sc in ((eg2_sb, 1.0), (e022_sb, ALPHA),
                             (e082_sb, 1.0 - ALPHA)):
                nc.scalar.activation(
                    e_sb[:, lo:hi].unsqueeze(2), gcols2, AF.Exp, scale=sc)
        for si, g in enumerate(S_GROUPS):
            for c in range(G):
                jc = g * G + c
                k = si * G + c
                nc.vector.tensor_scalar(
                    whb2t_sb[:, k * AUG1:(k + 1) * AUG1],
                    whb2_ch[:, jc, 0:AUG1],
                    e022_sb[:, jc:jc + 1], None, AT.mult)

        # ---- phase E: layer 2 ---- #
        pout2, frep2 = unit_start(fgb2_d[0:1, :])
        mm2 = [0, MM_TOTAL]
        args2 = (
            lambda jc: whb2_ch[:, jc, 0:AUG1],
            lambda k: whb2t_sb[:, k * AUG1:(k + 1) * AUG1],
            lambda jc: eg2_sb[:, jc:jc + 1],
            lambda jc: e022_sb[:, jc:jc + 1],
            lambda jc: e082_sb[:, jc:jc + 1],
        )
        for g in range(NG):
            unit_group(H, pout2, frep2, g, mm2, *args2)
        res2 = res.tile([D, R], F32, tag="res2")
        epilogue(pout2, res2[:], dst_f32=True)
        for ib in range(4):
            pt = psum.tile([128, D], F32, tag="bank")
            nc.tensor.transpose(
                pt[:], res2[:, ib * 128:(ib + 1) * 128], ident_sb[:])
            nc.vector.tensor_copy(out_sb[:, ib * D:(ib + 1) * D], pt[:])
        nc.sync.dma_start(
            out.ap().rearrange("(c p) w -> p c w", p=128),
            chunked_sb(out_sb[:], D))


# --------------------------------------------------------------------------- #
# host side
# --------------------------------------------------------------------------- #

def _pack_inputs(x, adj, W_heads, a_src, a_dst, W_out, a_src_out, a_dst_out):
    """Shard + repack the full inputs into the 8 per-core input maps."""
    x = np.asarray(x, np.float32)
    adj = np.asarray(adj)
    W_heads = np.asarray(W_heads, np.float32)
    a_src = np.asarray(a_src, np.float32)
    a_dst = np.asarray(a_dst, np.float32)
    W_out_np = np.asarray(W_out, np.float32)
    a_src_out = np.asarray(a_src_out, np.float32)
    a_dst_out = np.asarray(a_dst_out, np.float32)

    f16 = NPF16
    W_all = np.ascontiguousarray(
        W_heads.transpose(1, 0, 2).reshape(K, DALL)).astype(f16)     # [K, H*D]
    wa_cols = []
    for h in range(H):
        wa_cols.append(W_heads[h] @ a_src[h])
        wa_cols.append(W_heads[h] @ a_dst[h])
    wa = np.stack(wa_cols, axis=1).astype(f16)                       # [K, 16]
    W_out_p = W_out_np.astype(f16)                                   # [DALL, D]
    wa2 = np.stack([W_out_np @ a_src_out, W_out_np @ a_dst_out],
                   axis=1).astype(f16)                               # [DALL, 2]

    in_maps = []
    for c in range(NCORES):
        rows = slice(c * R, (c + 1) * R)
        adj_rows = (adj[rows, :] > 0).astype(np.float32)             # [R, N]
        adjB = np.ascontiguousarray(adj_rows.T).astype(f16)          # [N, R] 0/1
        in_maps.append({
            "xrT": np.ascontiguousarray(x[rows].T).astype(f16),
            "adjB": adjB,
            "W_all": W_all,
            "wa": wa,
            "W_out": W_out_p,
            "wa2": wa2,
        })
    return in_maps


def kernel(**inputs) -> np.ndarray:
    if "nc" not in _CACHE:
        _CACHE["nc"] = _build(emulate_collective=False)
    nc = _CACHE["nc"]
    in_maps = _pack_inputs(**inputs)
    res = run_bass_kernel_spmd(nc, in_maps, core_ids=list(range(NCORES)))
    return np.concatenate([res.results[c]["out"] for c in range(NCORES)], axis=0)


# revision 4
# speedup vs baseline: 1.4081x; 1.2144x over previous
"""Trainium2 Bass kernel for a 2-layer dense GAT (nn_GAT_87144886436203).

Sharding: row-shard the N=4096 nodes across 8 NeuronCores (512 rows each).
Each core computes attention for its row block against all N columns, with the
contraction axis j on SBUF partitions so `att @ Wh` needs no transposes.

Score factorization: with s = f_i + g_j and alpha = 0.2,
    exp(leaky_relu(s)) = exp(alpha*s) * max(exp((1-alpha)*s), 1)
                       = [exp(alpha*f_i)] * exp(alpha*g_j) * max(G_i * H_j, 1)
where G = exp(0.8 f), H = exp(0.8 g).  The exp(alpha*f_i) factor is constant
per attention row and cancels in the softmax normalization, so it is dropped.
The whole [N, N] exp/leaky-relu work collapses to O(N) vector exps plus, per
[128, 512] score tile, ONE DVE tensor_scalar (4x perf mode)
    C~ = (G_rep * e^{g_j}) max e^{0.2 g_j}   (= e^{0.2 g_j} * max(G H, 1))
and one tensor_tensor mask multiply by the 0/1 adjacency per group.  The
row-sum (softmax denominator) rides a ones-column through the same matmul.

Engine balance knobs: S_GROUPS run the C-build on ACT instead of DVE as
    Cm1 = Prelu_0(H_j*F'_j * G_rep - F'_j') ... actually Relu(G*H - 1) with
    F'-prescaled weights Whb~ = e^{0.2 g_j} * [Wh | 1], which needs one extra
    correction matmul  pout += Whb~^T @ adj  per chunk (the "+1").
POOL_COUNT masks run on GPSIMD instead of DVE.

Wh is computed per-core for OWN rows only and all-gathered as
[Wh_h|1]x8 | f,g  (536 cols fp16) -- the same collective pattern as the
layer-2 gather -- instead of every core redoing the full [4096, 512] matmul.
"""

import numpy as np
import ml_dtypes

import concourse.bass as bass
import concourse.bacc as bacc
import concourse.tile as tile
import concourse.mybir as mybir
from concourse import masks
from concourse.bass_utils import run_bass_kernel_spmd

F16 = mybir.dt.float16
F32 = mybir.dt.float32
NPF16 = ml_dtypes.float16 if hasattr(ml_dtypes, "float16") else np.float16

NCORES = 8
N = 4096            # nodes
K = 512             # input feature dim (= NFEAT)
H = 8               # heads (layer 1)
D = 64              # per-head hidden (= NHID = NCLASS)
DALL = H * D        # 512
R = N // NCORES     # 512 rows per core
JC = N // 128       # 32 j-chunks
G = 4               # j-chunks per group (free dim 2048 for the mask tt)
NG = JC // G        # 8 groups
AUG1 = D + 1        # 65: [Wh_h | ones]
W1S = H * AUG1      # 520: [Wh_h|1]x8 per-chunk width
CW1 = W1S + 2 * H   # 536: gathered layer-1 row payload [Wh|1]x8 | f,g
CW2 = D + 2         # 66: gathered layer-2 payload [Wh2 | 1 | g2]
ALPHA = 0.2
N_UNITS = H + 1     # 8 heads + layer-2

# ---- engine-balance knobs ---------------------------------------------- #
S_GROUPS = (1, 3, 5, 7)   # group indices whose C-build runs on ACT
POOL_COUNT = 0           # of the 72 (unit, group) masks, run this many on Pool
NS = len(S_GROUPS)


def _bres(i, count, total):
    return (i * count) // total != ((i + 1) * count) // total


def _mask_on_pool(unit, g):
    return _bres(unit * NG + g, POOL_COUNT, N_UNITS * NG)


_CACHE = {}


# --------------------------------------------------------------------------- #
# device program
# --------------------------------------------------------------------------- #

def _build(emulate_collective=False):
    nc = bacc.Bacc(
        "TRN2",
        target_bir_lowering=False,
        debug=False,
        num_devices=1 if emulate_collective else NCORES,
    )

    xrT = nc.dram_tensor("xrT", [K, R], F16, kind="ExternalInput")
    adjB = nc.dram_tensor("adjB", [N, R], F16, kind="ExternalInput")
    W_all = nc.dram_tensor("W_all", [K, DALL], F16, kind="ExternalInput")
    wa = nc.dram_tensor("wa", [K, 2 * H], F16, kind="ExternalInput")
    W_out = nc.dram_tensor("W_out", [DALL, D], F16, kind="ExternalInput")
    wa2 = nc.dram_tensor("wa2", [DALL, 2], F16, kind="ExternalInput")
    out = nc.dram_tensor("out", [R, D], F32, kind="ExternalOutput")

    with tile.TileContext(nc) as tc:
        _emit(nc, tc, locals(), emulate_collective)

    nc.compile()
    return nc


def _emit(nc, tc, io, emulate_collective):
    xrT, adjB, W_all, wa, W_out, wa2, out = (
        io["xrT"], io["adjB"], io["W_all"], io["wa"],
        io["W_out"], io["wa2"], io["out"],
    )
    AT = mybir.AluOpType
    AF = mybir.ActivationFunctionType

    from contextlib import ExitStack
    with ExitStack() as ctx:
        res = ctx.enter_context(tc.tile_pool(name="res", bufs=1))
        psum = ctx.enter_context(tc.tile_pool(name="psum", bufs=4, space="PSUM"))
        ppool = ctx.enter_context(tc.tile_pool(name="ppool", bufs=4, space="PSUM"))
        work = ctx.enter_context(tc.tile_pool(name="work", bufs=3))
        work2 = ctx.enter_context(tc.tile_pool(name="work2", bufs=3))
        tpool = ctx.enter_context(tc.tile_pool(name="tpool", bufs=4))
        small = ctx.enter_context(tc.tile_pool(name="small", bufs=4))
        rpool = ctx.enter_context(tc.tile_pool(name="rpool", bufs=2))
        dram = ctx.enter_context(tc.tile_pool(name="dram", bufs=1, space="DRAM"))

        # ---- resident SBUF tensors ---- #
        xrT_sb = res.tile([128, 4 * R], F16, tag="xrT")
        adjB_sb = res.tile([128, JC * R], F16, tag="adjB")
        W_all_sb = res.tile([128, 4 * DALL], F16, tag="W_all")
        wa_sb = res.tile([128, 4 * 2 * H], F16, tag="wa")
        W_out_sb = res.tile([128, 4 * D], F16, tag="W_out")
        wa2_sb = res.tile([128, 4 * 2], F16, tag="wa2")
        whbig_sb = res.tile([128, JC * CW1], F16, tag="whbig")  # gathered L1
        whbt_sb = res.tile([128, NS * G * W1S], F16, tag="whbt")  # F'-scaled
        eg_sb = res.tile([128, JC * H], F32, tag="eg")     # exp(g)
        e02_sb = res.tile([128, JC * H], F32, tag="e02")   # exp(0.2 g)
        e08_sb = res.tile([128, JC * H], F32, tag="e08")   # exp(0.8 g)
        e02h_sb = res.tile([128, JC * H], F16, tag="e02h")
        hcatT_sb = res.tile([128, 4 * R], F16, tag="hcatT")
        whb2_sb = res.tile([128, JC * CW2], F16, tag="whb2")
        whb2t_sb = res.tile([128, NS * G * AUG1], F16, tag="whb2t")
        eg2_sb = res.tile([128, JC], F32, tag="eg2")
        e022_sb = res.tile([128, JC], F32, tag="e022")
        e082_sb = res.tile([128, JC], F32, tag="e082")
        ones_sb = res.tile([1, 128], F32, tag="ones")
        neg1_sb = res.tile([128, 1], F32, tag="neg1")
        ident_sb = res.tile([64, 64], F32, tag="ident")
        out_sb = res.tile([128, 4 * D], F32, tag="out_sb")

        def chunked(dram_t, width):
            return dram_t.ap().rearrange("(c p) w -> p c w", p=128)

        def chunked_sb(sb_ap, width):
            return sb_ap.rearrange("p (c w) -> p c w", w=width)

        def load(sb_tile, dram_t, width, split=1, split_free=1):
            dst = chunked_sb(sb_tile[:], width)
            src = chunked(dram_t, width)
            nch = dst.shape[1]
            step = max(1, nch // split)
            fstep = max(1, width // split_free)
            for lo in range(0, nch, step):
                hi = min(nch, lo + step)
                for flo in range(0, width, fstep):
                    fhi = min(width, flo + fstep)
                    nc.sync.dma_start(
                        dst[:, lo:hi, flo:fhi], src[:, lo:hi, flo:fhi])

        # ---- phase 0: loads + constants ---- #
        load(xrT_sb, xrT, R)
        load(wa_sb, wa, 2 * H)
        load(W_all_sb, W_all, DALL)
        load(adjB_sb, adjB, R, split=8)
        load(W_out_sb, W_out, D)
        load(wa2_sb, wa2, 2)
        nc.vector.memset(ones_sb[:], 1.0)
        nc.vector.memset(neg1_sb[:], -1.0)
        masks.make_identity(nc, ident_sb[:])

        whbig_ch = chunked_sb(whbig_sb[:], CW1)
        whb2_ch = chunked_sb(whb2_sb[:], CW2)

        # ---- phase A: own-row Wh/f/g + G-row, then allgather ---- #
        # G-row bounce: rows 2h hold exp(0.8 * f_h) for the frep broadcasts.
        pfr = psum.tile([16, R], F32, tag="bank")
        for kc in range(4):
            nc.tensor.matmul(
                pfr[:], wa_sb[:, kc * 2 * H:(kc + 1) * 2 * H],
                xrT_sb[:, kc * R:(kc + 1) * R],
                start=(kc == 0), stop=(kc == 3),
            )
        gx16 = res.tile([16, R], F16, tag="gx16")
        nc.scalar.activation(gx16[:], pfr[:], AF.Exp, scale=1.0 - ALPHA)
        fgb_d = dram.tile([16, R], F16, tag="fgb")
        nc.gpsimd.dma_start(fgb_d[:], gx16[:])

        gt1 = res.tile([128, 4 * CW1], F16, tag="gt1")
        nc.gpsimd.memset(gt1[:], 1.0)   # bakes the ones columns
        for ib in range(4):
            pw = psum.tile([128, DALL], F32, tag="bank")
            pf = psum.tile([128, 2 * H], F32, tag="bank")
            for kc in range(4):
                lhsT = xrT_sb[:, kc * R + ib * 128: kc * R + (ib + 1) * 128]
                nc.tensor.matmul(
                    pw[:], lhsT, W_all_sb[:, kc * DALL:(kc + 1) * DALL],
                    start=(kc == 0), stop=(kc == 3))
                nc.tensor.matmul(
                    pf[:], lhsT, wa_sb[:, kc * 2 * H:(kc + 1) * 2 * H],
                    start=(kc == 0), stop=(kc == 3))
            dst = gt1[:, ib * CW1: ib * CW1 + W1S].rearrange(
                "p (h x) -> p h x", x=AUG1)[:, :, 0:D]
            nc.scalar.activation(
                dst, pw.rearrange("p (h x) -> p h x", x=D), AF.Copy)
            nc.vector.tensor_copy(
                gt1[:, ib * CW1 + W1S:(ib + 1) * CW1], pf[:])

        cc1_in = dram.tile([R, CW1], F16, tag="cc1_in")
        cc_space = {} if emulate_collective else {"addr_space": "Shared"}
        cc1_out = dram.tile([N, CW1], F16, tag="cc1_out", **cc_space)
        nc.sync.dma_start(
            cc1_in[:].rearrange("(c p) w -> p c w", p=128),
            chunked_sb(gt1[:], CW1))
        if emulate_collective:
            for c in range(NCORES):
                nc.sync.dma_start(cc1_out[c * R:(c + 1) * R, :], cc1_in[:])
        else:
            nc.gpsimd.collective_compute(
                "AllGather", mybir.AluOpType.bypass,
                replica_groups=[list(range(NCORES))],
                ins=[cc1_in.opt()], outs=[cc1_out.opt()],
            )

        # ---- phase B: land the gather + per-node exp vectors ---- #
        cc1_out_ch = cc1_out[:].rearrange("(c p) w -> p c w", p=128)
        for half in range(2):
            lo, hi = half * (JC // 2), (half + 1) * (JC // 2)
            nc.sync.dma_start(whbig_ch[:, lo:hi, :], cc1_out_ch[:, lo:hi, :])
            gcols = whbig_ch[:, lo:hi, W1S:CW1].rearrange(
                "p c (h two) -> p c h two", two=2)[:, :, :, 1:2]
            for e_sb, sc in ((eg_sb, 1.0), (e02_sb, ALPHA), (e08_sb, 1.0 - ALPHA)):
                dst = e_sb[:, lo * H:hi * H].rearrange(
                    "p (c h) -> p c h", h=H).unsqueeze(3)
                nc.scalar.activation(dst, gcols, AF.Exp, scale=sc)
            nc.vector.tensor_copy(
                e02h_sb[:, lo * H:hi * H], e02_sb[:, lo * H:hi * H])

        # F'-scaled weights for the ACT-built chunks (all 8 heads at once)
        for si, g in enumerate(S_GROUPS):
            for c in range(G):
                jc = g * G + c
                k = si * G + c
                src = whbig_ch[:, jc, 0:W1S].rearrange(
                    "p (h x) -> p h x", x=AUG1)
                fb = e02h_sb[:, jc * H:(jc + 1) * H].unsqueeze(2) \
                    .broadcast_to([128, H, AUG1])
                nc.gpsimd.tensor_tensor(
                    whbt_sb[:, k * W1S:(k + 1) * W1S].rearrange(
                        "p (h x) -> p h x", x=AUG1),
                    src, fb, AT.mult)

        # ---- attention unit ---- #
        def unit_start(f_row_dram):
            pout = ppool.tile([AUG1, R], F32, tag="pout")
            frep = tpool.tile([128, R], F16, tag="frep")
            nc.gpsimd.dma_start(frep[:], f_row_dram.broadcast_to([128, R]))
            return pout, frep

        def unit_group(unit, pout, frep, g, mm, lhsT_of, lhsTs_of,
                       eg_of, e02_of, e08_of):
            """mm = [next_idx, total]; returns updated mm."""
            on_act = g in S_GROUPS
            si = S_GROUPS.index(g) if on_act else None
            u = work.tile([128, G * R], F16, tag="u")
            for c in range(G):
                jc = g * G + c
                if on_act:
                    nc.scalar.activation(
                        u[:, c * R:(c + 1) * R], frep[:],
                        AF.Prelu, bias=neg1_sb[:], scale=e08_of(jc),
                        alpha=0.0)
                else:
                    nc.vector.tensor_scalar(
                        u[:, c * R:(c + 1) * R], frep[:],
                        eg_of(jc), e02_of(jc), AT.mult, AT.max)
            pm = work2.tile([128, G * R], F16, tag="pm")
            eng = nc.gpsimd if _mask_on_pool(unit, g) else nc.vector
            eng.tensor_tensor(
                pm[:], u[:], adjB_sb[:, g * G * R:(g + 1) * G * R], AT.mult)
            for c in range(G):
                jc = g * G + c
                lhsT = lhsTs_of(si * G + c) if on_act else lhsT_of(jc)
                nc.tensor.matmul(
                    pout[:], lhsT, pm[:, c * R:(c + 1) * R],
                    start=(mm[0] == 0), stop=(mm[0] == mm[1] - 1))
                mm[0] += 1
            if on_act:
                for c in range(G):
                    jc = g * G + c
                    nc.tensor.matmul(
                        pout[:], lhsTs_of(si * G + c),
                        adjB_sb[:, jc * R:(jc + 1) * R],
                        start=(mm[0] == 0), stop=(mm[0] == mm[1] - 1))
                    mm[0] += 1

        def epilogue(pout, dst_ap, dst_f32):
            """dst = elu(att_out / rowsum) written to dst_ap ([64, R])."""
            dt = F32 if dst_f32 else F16
            recip = rpool.tile([1, R], F32, tag="recip")
            nc.vector.reciprocal(recip[:], pout[D:D + 1, :])
            pr = psum.tile([D, R], F32, tag="bank")
            nc.tensor.matmul(pr[:], ones_sb[0:1, 0:D], recip[:])
            rsb = small.tile([D, R], F32, tag="ep")
            nc.any.tensor_copy(rsb[:], pr[:])
            hl = small.tile([D, R], dt, tag="ep")
            nc.vector.tensor_tensor(hl[:], pout[0:D, :], rsb[:], AT.mult)
            # elu(x) = max(x,0) + min(exp(x),1) - 1   (exp monotone)
            q = small.tile([D, R], dt, tag="ep")
            nc.scalar.activation(q[:], hl[:], AF.Exp)
            t1 = small.tile([D, R], dt, tag="ep")
            nc.vector.tensor_scalar(t1[:], q[:], 1.0, -1.0, AT.min, AT.add)
            t2 = small.tile([D, R], dt, tag="ep")
            nc.vector.tensor_scalar(t2[:], hl[:], 0.0, None, AT.max)
            nc.vector.tensor_tensor(dst_ap, t1[:], t2[:], AT.add)

        # ---- phase C: layer-1 heads, two at a time ---- #
        MM_TOTAL = JC + NS * G

        def l1_args(h):
            return (
                lambda jc, h=h: whbig_ch[:, jc, h * AUG1:(h + 1) * AUG1],
                lambda k, h=h: whbt_sb[:, k * W1S + h * AUG1:
                                       k * W1S + (h + 1) * AUG1],
                lambda jc, h=h: eg_sb[:, jc * H + h: jc * H + h + 1],
                lambda jc, h=h: e02_sb[:, jc * H + h: jc * H + h + 1],
                lambda jc, h=h: e08_sb[:, jc * H + h: jc * H + h + 1],
            )

        for hp in range(0, H, 2):
            pair = []
            for h in (hp, hp + 1):
                pout, frep = unit_start(fgb_d[2 * h:2 * h + 1, :])
                pair.append([h, pout, frep, [0, MM_TOTAL], l1_args(h)])
            for gi in range(NG):
                for pi, (h, pout, frep, mm, args) in enumerate(pair):
                    # stagger the pair by one group so one unit is in an
                    # ACT-built group while the other is in a DVE-built one
                    unit_group(h, pout, frep, (gi + pi) % NG, mm, *args)
            for (h, pout, frep, mm, args) in pair:
                kc, po = h // 2, (h % 2) * D
                epilogue(pout, hcatT_sb[po:po + D, kc * R:(kc + 1) * R],
                         dst_f32=False)

        # ---- phase D: layer-2 prep + allgather ---- #
        gt2 = res.tile([128, 4 * CW2], F16, tag="gt2")
        nc.vector.memset(gt2[:], 1.0)
        for ib in range(4):
            pw2 = psum.tile([128, D], F32, tag="bank")
            pg2 = psum.tile([128, 2], F32, tag="bank")
            for kc in range(4):
                lhsT = hcatT_sb[:, kc * R + ib * 128: kc * R + (ib + 1) * 128]
                nc.tensor.matmul(pw2[:], lhsT, W_out_sb[:, kc * D:(kc + 1) * D],
                                 start=(kc == 0), stop=(kc == 3))
                nc.tensor.matmul(pg2[:], lhsT, wa2_sb[:, kc * 2:(kc + 1) * 2],
                                 start=(kc == 0), stop=(kc == 3))
            nc.vector.tensor_copy(gt2[:, ib * CW2: ib * CW2 + D], pw2[:])
            nc.vector.tensor_copy(
                gt2[:, ib * CW2 + D + 1: ib * CW2 + D + 2], pg2[:, 1:2])

        pfg2 = psum.tile([2, R], F32, tag="bank")
        for kc in range(4):
            nc.tensor.matmul(pfg2[:], wa2_sb[:, kc * 2:(kc + 1) * 2],
                             hcatT_sb[:, kc * R:(kc + 1) * R],
                             start=(kc == 0), stop=(kc == 3))
        g2row = res.tile([1, R], F16, tag="g2row")
        nc.scalar.activation(g2row[:], pfg2[0:1, :], AF.Exp, scale=1.0 - ALPHA)
        fgb2_d = dram.tile([1, R], F16, tag="fgb2")
        nc.sync.dma_start(fgb2_d[:], g2row[:])

        cc2_in = dram.tile([R, CW2], F16, tag="cc2_in")
        cc2_out = dram.tile([N, CW2], F16, tag="cc2_out", **cc_space)
        nc.sync.dma_start(
            cc2_in[:].rearrange("(c p) w -> p c w", p=128),
            chunked_sb(gt2[:], CW2))
        if emulate_collective:
            for c in range(NCORES):
                nc.sync.dma_start(cc2_out[c * R:(c + 1) * R, :], cc2_in[:])
        else:
            nc.gpsimd.collective_compute(
                "AllGather", mybir.AluOpType.bypass,
                replica_groups=[list(range(NCORES))],
                ins=[cc2_in.opt()], outs=[cc2_out.opt()],
            )
        cc2_out_ch = cc2_out[:].rearrange("(c p) w -> p c w", p=128)
        for half in range(2):
            lo, hi = half * (JC // 2), (half + 1) * (JC // 2)
            nc.sync.dma_start(whb2_ch[:, lo:hi, :], cc2_out_ch[:, lo:hi, :])
            gcols2 = whb2_ch[:, lo:hi, D + 1:D + 2]
            for e_sb, sc in ((eg2_sb, 1.0), (e022_sb, ALPHA),
                             (e082_sb, 1.0 - ALPHA)):
                nc.scalar.activation(
                    e_sb[:, lo:hi].unsqueeze(2), gcols2, AF.Exp, scale=sc)
        for si, g in enumerate(S_GROUPS):
            for c in range(G):
                jc = g * G + c
                k = si * G + c
                nc.vector.tensor_scalar(
                    whb2t_sb[:, k * AUG1:(k + 1) * AUG1],
                    whb2_ch[:, jc, 0:AUG1],
                    e022_sb[:, jc:jc + 1], None, AT.mult)

        # ---- phase E: layer 2 ---- #
        pout2, frep2 = unit_start(fgb2_d[0:1, :])
        mm2 = [0, MM_TOTAL]
        args2 = (
            lambda jc: whb2_ch[:, jc, 0:AUG1],
            lambda k: whb2t_sb[:, k * AUG1:(k + 1) * AUG1],
            lambda jc: eg2_sb[:, jc:jc + 1],
            lambda jc: e022_sb[:, jc:jc + 1],
            lambda jc: e082_sb[:, jc:jc + 1],
        )
        for g in range(NG):
            unit_group(H, pout2, frep2, g, mm2, *args2)
        res2 = res.tile([D, R], F32, tag="res2")
        epilogue(pout2, res2[:], dst_f32=True)
        for ib in range(4):
            pt = psum.tile([128, D], F32, tag="bank")
            nc.tensor.transpose(
                pt[:], res2[:, ib * 128:(ib + 1) * 128], ident_sb[:])
            nc.vector.tensor_copy(out_sb[:, ib * D:(ib + 1) * D], pt[:])
        nc.sync.dma_start(
            out.ap().rearrange("(c p) w -> p c w", p=128),
            chunked_sb(out_sb[:], D))


# --------------------------------------------------------------------------- #
# host side
# --------------------------------------------------------------------------- #

def _pack_inputs(x, adj, W_heads, a_src, a_dst, W_out, a_src_out, a_dst_out):
    """Shard + repack the full inputs into the 8 per-core input maps."""
    x = np.asarray(x, np.float32)
    adj = np.asarray(adj)
    W_heads = np.asarray(W_heads, np.float32)
    a_src = np.asarray(a_src, np.float32)
    a_dst = np.asarray(a_dst, np.float32)
    W_out_np = np.asarray(W_out, np.float32)
    a_src_out = np.asarray(a_src_out, np.float32)
    a_dst_out = np.asarray(a_dst_out, np.float32)

    f16 = NPF16
    W_all = np.ascontiguousarray(
        W_heads.transpose(1, 0, 2).reshape(K, DALL)).astype(f16)     # [K, H*D]
    wa_cols = []
    for h in range(H):
        wa_cols.append(W_heads[h] @ a_src[h])
        wa_cols.append(W_heads[h] @ a_dst[h])
    wa = np.stack(wa_cols, axis=1).astype(f16)                       # [K, 16]
    W_out_p = W_out_np.astype(f16)                                   # [DALL, D]
    wa2 = np.stack([W_out_np @ a_src_out, W_out_np @ a_dst_out],
                   axis=1).astype(f16)                               # [DALL, 2]

    in_maps = []
    for c in range(NCORES):
        rows = slice(c * R, (c + 1) * R)
        adj_rows = (adj[rows, :] > 0).astype(np.float32)             # [R, N]
        adjB = np.ascontiguousarray(adj_rows.T).astype(f16)          # [N, R] 0/1
        in_maps.append({
            "xrT": np.ascontiguousarray(x[rows].T).astype(f16),
            "adjB": adjB,
            "W_all": W_all,
            "wa": wa,
            "W_out": W_out_p,
            "wa2": wa2,
        })
    return in_maps


def kernel(**inputs) -> np.ndarray:
    if "nc" not in _CACHE:
        _CACHE["nc"] = _build(emulate_collective=False)
    nc = _CACHE["nc"]
    in_maps = _pack_inputs(**inputs)
    res = run_bass_kernel_spmd(nc, in_maps, core_ids=list(range(NCORES)))
    return np.concatenate([res.results[c]["out"] for c in range(NCORES)], axis=0)
